# revision 1
# baseline (speedup 1.0000x reference)
"""Trainium2 Bass kernel for nn_GCLMemory (content-addressed memory read weights).

Full computation per batch sample b:
    dots[n]  = <keys[b,n,:], k[b,:]>
    cos[n]   = dots[n] / (max(||keys[b,n]||,eps) * max(||k[b]||,eps))
    wc       = softmax(beta[b] * cos)
    top-32 mask (1.0 at top-32 of wc, 1e-16 elsewhere), renormalize,
    w        = wc**gamma[b], renormalize.

Sharding: data-parallel over batch. 8 cores x 16 samples. Host pre-transposes
each core's keys slice to [2, 128, 16, 2048] (K on partitions) so the PE can
contract over K. Per (sample b, k-chunk q) the kernel streams a [128, 2048]
tile, squares it on ACT, and issues matmuls with lhsT = kvec_b (dots) and
lhsT = ones (row sumsq), accumulating into PSUM rows at partition offset b,
which yields row-major [16, 512] dots/sumsq tiles with no transposes.

Tail identity used: the intermediate renormalizations cancel, so
    w = em**gamma / sum(em**gamma),  em = e*1 at top-32, e*1e-16 elsewhere,
    e = exp(beta*cos)   (no max-subtraction needed: |beta*cos| <= ~5.5).
Top-32 found with 4 rounds of DVE max8 + match_replace(0.0) on a copy of e;
the zeroed copy provides the mask via  em = (e - e_rem) + 1e-16*e_rem.
"""

import sys

import numpy as np

sys.path.insert(0, "/opt/trn_rl_repo")

import concourse.bass as bass
import concourse.mybir as mybir
from concourse.bass_utils import run_bass_kernel_spmd
from concourse.tile import TileContext
from concourse import masks

F32 = mybir.dt.float32
Alu = mybir.AluOpType
Act = mybir.ActivationFunctionType

# ---------------------------------------------------------------------------
# This container's walrus build only accepts a single sem-wait command per
# instruction ("Too many sync wait commands" in CoreV3GenImpl otherwise), but
# Tile's exit drain aggregates one wait per busy processor. Split any
# multi-wait instruction into preceding single-wait Drains on the same engine.
# ---------------------------------------------------------------------------
_WAIT_LIMIT = 1


def _split_multi_waits(bir_bytes: bytes, limit: int = _WAIT_LIMIT) -> bytes:
    import orjson
    d = orjson.loads(bir_bytes)
    n_split = 0
    for fn in d.get("functions", []):
        for bb in fn.get("blocks", []):
            out = []
            for inst in bb.get("instructions", []):
                si = inst.get("sync_info") or {}
                waits = si.get("on_wait") or []
                if len(waits) > limit:
                    n_split += 1
                    chunks = [waits[i:i + limit]
                              for i in range(0, len(waits), limit)]
                    for j, ch in enumerate(chunks[:-1]):
                        carrier = {
                            "engine": inst["engine"],
                            "ins": [],
                            "is_reset_sema": False,
                            "name": f"{inst['name']}__w{j}",
                            "opcode": "Drain",
                            "outs": [],
                            "sync_info": {"on_update": [], "on_wait": ch},
                        }
                        if "debug" in inst:
                            carrier["debug"] = inst["debug"]
                        out.append(carrier)
                    si["on_wait"] = chunks[-1]
                out.append(inst)
            bb["instructions"] = out
    return orjson.dumps(d)


def _install_wait_split_hook():
    from concourse import bass2jax
    orig = bass2jax.compile_bir_kernel
    if getattr(orig, "_wait_split_wrapped", False):
        return

    def wrapped(bir_bytes, *args, **kwargs):
        return orig(_split_multi_waits(bir_bytes), *args, **kwargs)

    wrapped._wait_split_wrapped = True
    bass2jax.compile_bir_kernel = wrapped


_install_wait_split_hook()

B, N, K = 128, 2048, 256
M = 8            # cores
BPC = B // M     # samples per core
KQ = K // 128    # contraction chunks
NT = N // 512    # psum column tiles
CAND = 32
EPS = 1e-8


def build_nc(n_reps: int = 1, tail: bool = True):
    nc = bass.Bass()
    keysT = nc.declare_dram_parameter("keysT", [KQ, 128, BPC, N], F32, isOutput=False)
    kvT17 = nc.declare_dram_parameter("kvT17", [KQ, 128, 32], F32, isOutput=False)
    kvr = nc.declare_dram_parameter("kvr", [BPC, K], F32, isOutput=False)
    beta = nc.declare_dram_parameter("beta", [BPC, 1], F32, isOutput=False)
    gamma = nc.declare_dram_parameter("gamma", [BPC, 1], F32, isOutput=False)
    out = nc.declare_dram_parameter("out", [BPC, N], F32, isOutput=True)

    with TileContext(nc) as tc:
        with (
            tc.tile_pool(name="const", bufs=1) as cpool,
            tc.tile_pool(name="stream", bufs=3) as spool,
            tc.tile_pool(name="psum", bufs=1, space="PSUM") as ppool,
        ):
          for _rep in range(n_reps):
              ones = cpool.tile([128, 1], F32, tag="ones")
              nc.vector.memset(ones[:], 1.0)
              kv = []
              for q in range(KQ):
                  t = cpool.tile([128, 32], F32, name=f"kv{q}", tag=f"kv{q}")
                  nc.sync.dma_start(out=t[:], in_=kvT17[q])
                  kv.append(t)
              kvr_t = cpool.tile([BPC, K], F32, tag="kvr")
              nc.sync.dma_start(out=kvr_t[:], in_=kvr[:])
              beta_t = cpool.tile([BPC, 1], F32, tag="beta")
              nc.sync.dma_start(out=beta_t[:], in_=beta[:])
              gamma_t = cpool.tile([BPC, 1], F32, tag="gamma")
              nc.sync.dma_start(out=gamma_t[:], in_=gamma[:])

              # lnscale = ln(beta) - 0.5*ln(||k||^2): avoids sqrt+reciprocal
              # (InstReciprocal is ~13us on this platform).
              qsq = cpool.tile([BPC, K], F32, tag="qsq")
              qn2 = cpool.tile([BPC, 1], F32, tag="qn2")
              nc.scalar.activation(qsq[:], kvr_t[:], Act.Square,
                                   accum_out=qn2[:])
              lnb = cpool.tile([BPC, 1], F32, tag="lnb")
              nc.scalar.activation(lnb[:], beta_t[:], Act.Ln)
              lnq2 = cpool.tile([BPC, 1], F32, tag="lnq2")
              nc.scalar.activation(lnq2[:], qn2[:], Act.Ln)
              lnscale = cpool.tile([BPC, 1], F32, tag="lnscale")
              nc.vector.scalar_tensor_tensor(
                  lnscale[:], lnq2[:], -0.5, lnb[:],
                  op0=Alu.mult, op1=Alu.add)

              D = cpool.tile([BPC, N], F32, tag="D")
              S = cpool.tile([BPC, N], F32, tag="S")
              # Row-orientation stream: stationary lhsT = [kv_0..kv_15 | ones]
              # [128, 17]; rhs = 512-col slices of raw (dots) or squared
              # (sumsq) stream tiles. Each matmul's useful output row is b
              # (dots, raw stream) or 16 (sumsq, sq stream); jobs are packed
              # 3-per-PSUM-tile at base partitions 0/32/64 (the only legal
              # bases), staged to SBUF with one [81,512] copy, and the useful
              # rows land in D/S via tiny SBUF->SBUF row DMAs.
              GRP = 2  # samples per stream tile: 16KB DMA lines
              jobs = []
              for g in range(BPC // GRP):
                  for j in range(GRP):
                      for kind in range(2):
                          jobs.append((g, j, kind))
              stream_tiles = {}
              cur = None
              cur_jobs = []
              slot = 3
              tile_i = 0

              def flush_tile():
                  nrows = 32 * len(cur_jobs)
                  stag = spool.tile([96, N], F32, name=f"stag{tile_i}",
                                    tag="stag", bufs=2)
                  nc.vector.tensor_copy(stag[0:nrows, :], cur[0:nrows, :])
                  for (s_, b_, kind_) in cur_jobs:
                      row = 32 * s_ + (b_ if kind_ == 0 else 16)
                      dst = D if kind_ == 0 else S
                      nc.sync.dma_start(out=dst[b_:b_ + 1, :],
                                        in_=stag[row:row + 1, :])

              for (g, j, kind) in jobs:
                  if g not in stream_tiles:
                      raws, sqs = [], []
                      for q in range(KQ):
                          raw = spool.tile([128, GRP * N], F32,
                                           name=f"raw{q}", tag=f"raw{q}",
                                           bufs=2)
                          nc.sync.dma_start(
                              out=raw[:],
                              in_=keysT[q, :, GRP * g:GRP * (g + 1), :])
                          sq = spool.tile([128, GRP * N], F32,
                                          name=f"sq{q}", tag=f"sq{q}", bufs=2)
                          nc.scalar.square(sq[:], raw[:])
                          raws.append(raw)
                          sqs.append(sq)
                      stream_tiles[g] = (raws, sqs)
                  raws, sqs = stream_tiles[g]
                  if slot == 3:
                      if cur is not None:
                          flush_tile()
                      tile_i += 1
                      cur = ppool.tile([96, N], F32, name=f"P{tile_i}",
                                       tag="P", bufs=2)
                      cur_jobs = []
                      slot = 0
                  src = raws if kind == 0 else sqs
                  for t in range(N // 512):
                      csl = slice(j * N + 512 * t, j * N + 512 * (t + 1))
                      for q in range(KQ):
                          nc.tensor.matmul(
                              cur[32 * slot:32 * slot + 32,
                                  512 * t:512 * (t + 1)],
                              kv[q][:, 0:32], src[q][:, csl],
                              start=(q == 0), stop=(q == KQ - 1))
                  cur_jobs.append((slot, GRP * g + j, kind))
                  slot += 1
              if cur is not None:
                  flush_tile()

              # ---- tail, all on [BPC, N] rows ----
              # rfold = exp(-0.5*ln(S) + lnscale) = beta/(||k||*||keys_n||)
              lnS = cpool.tile([BPC, N], F32, tag="t1", name="lnS")
              nc.scalar.activation(lnS[:], S[:], Act.Ln)
              rfold = cpool.tile([BPC, N], F32, tag="t2", name="rfold")
              nc.scalar.activation(rfold[:], lnS[:], Act.Exp, scale=-0.5,
                                   bias=lnscale[:])
              logits = cpool.tile([BPC, N], F32, tag="t1", name="logits")
              nc.vector.tensor_tensor(logits[:], D[:], rfold[:], Alu.mult)
              e1 = cpool.tile([BPC, N], F32, tag="t2", name="e1")
              nc.scalar.activation(e1[:], logits[:], Act.Exp)
              e2 = cpool.tile([BPC, N], F32, tag="t1", name="e2")
              nc.vector.tensor_copy(e2[:], e1[:])
              m8 = cpool.tile([BPC, 8], F32, tag="m8", name="m8")
              for rd in range(CAND // 8):
                  nc.vector.max(m8[:], e2[:])
                  if rd < CAND // 8 - 1:
                      nc.vector.match_replace(e2[:], m8[:], e2[:], 0.0)
              # em = etop + 1e-16*e1, etop = e1 * (e1 >= t32)
              etop = cpool.tile([BPC, N], F32, tag="t3", name="etop")
              nc.vector.scalar_tensor_tensor(
                  etop[:], e1[:], m8[:, 7:8], e1[:],
                  op0=Alu.is_ge, op1=Alu.mult)
              em = cpool.tile([BPC, N], F32, tag="t1", name="em")
              nc.vector.scalar_tensor_tensor(
                  em[:], e1[:], 1e-16, etop[:], op0=Alu.mult, op1=Alu.add)
              lgm = cpool.tile([BPC, N], F32, tag="t2", name="lgm")
              nc.scalar.activation(lgm[:], em[:], Act.Ln)
              wt = cpool.tile([BPC, N], F32, tag="t3", name="wt")
              zsum = cpool.tile([BPC, 1], F32, tag="zsum")
              nc.scalar.activation(wt[:], lgm[:], Act.Exp, scale=gamma_t[:],
                                   accum_out=zsum[:])
              zr = cpool.tile([BPC, 1], F32, tag="zr")
              nc.vector.reciprocal(zr[:], zsum[:])
              w = cpool.tile([BPC, N], F32, tag="t1", name="w")
              nc.vector.tensor_scalar(w[:], wt[:], zr[:], None, Alu.mult)
              nc.sync.dma_start(out=out[:], in_=w[:])
    return nc


def shard_inputs(k, beta, gamma, keys):
    k = np.ascontiguousarray(k, dtype=np.float32)
    beta = np.ascontiguousarray(beta, dtype=np.float32)
    gamma = np.ascontiguousarray(gamma, dtype=np.float32)
    keys = np.ascontiguousarray(keys, dtype=np.float32)
    in_maps = []
    for c in range(M):
        sl = slice(c * BPC, (c + 1) * BPC)
        kc = np.ascontiguousarray(k[sl])                       # [BPC, K]
        keysTc = np.ascontiguousarray(
            keys[sl].transpose(2, 0, 1)).reshape(KQ, 128, BPC, N)
        kvTc = np.ascontiguousarray(kc.T).reshape(KQ, 128, BPC)
        kvT17c = np.zeros((KQ, 128, 32), np.float32)
        kvT17c[:, :, 0:BPC] = kvTc
        kvT17c[:, :, 16] = 1.0
        in_maps.append({
            "keysT": keysTc,
            "kvT17": kvT17c,
            "kvr": kc,
            "beta": np.ascontiguousarray(beta[sl]),
            "gamma": np.ascontiguousarray(gamma[sl]),
        })
    return in_maps


_NC_CACHE = None


def kernel(k=None, beta=None, gamma=None, keys=None, candidates=None, **_ignored):
    assert int(candidates) == CAND, f"kernel hardcoded for candidates=32, got {candidates}"
    assert keys.shape == (B, N, K), keys.shape
    global _NC_CACHE
    if _NC_CACHE is None:
        _NC_CACHE = build_nc()
    in_maps = shard_inputs(k, beta, gamma, keys)
    res = run_bass_kernel_spmd(_NC_CACHE, in_maps, list(range(M))).results
    return np.concatenate([res[c]["out"] for c in range(M)], axis=0)



# revision 4
# speedup vs baseline: 1.9358x; 1.9358x over previous
"""Trainium2 Bass kernel for nn_GCLMemory (content-addressed memory read weights).

Per batch sample b:
    cos[n] = <keys[b,n], k[b]> / (||keys[b,n]|| * ||k[b]||)
    wc     = softmax(beta[b] * cos); top-32 mask; renorm; w = wc**gamma; renorm.

Sharding: data-parallel over batch, 8 cores x 16 samples.

Stream encoding (host-prepped): keys are shipped as an fp16 "hi" stream
(scaled x32) plus an fp8e4m3 residual stream that folds in BOTH the fp16
rounding error of the keys AND the fp16 rounding error of the query
(res = RS*(32*keys - hi + hi*kv_res/kv_hi)), so
    RS*HS*dots ~= <hi, RS*kv16> + <res8, kv16>
accumulates in a single PSUM row at near-fp32 accuracy (needed: the top-32
selection flips on ~1e-5 logit gaps, and one flipped row alone costs 2e-2
rel err). Row sumsq comes from an on-chip fp16 square of the hi stream
against a ones lhsT column. fp16/fp8 matmuls run at 1 cycle/row vs fp32's 4.

Tail runs in a segment layout [128, 256] (sample s = partitions 8s..8s+7)
so elementwise ops use all 128 lanes; the per-sample top-32 is found as
seg-top-32 (DVE max8/match_replace) -> DMA-gather to rows [16, 256] ->
final top-32; [16,1]<->[128,1] broadcasts/reductions go through tiny PE
matmuls with 0/1 matrices. Normalizations cancel except the final one:
w = (logits >= t32) * exp(gamma*logits) / sum.
"""

import sys

import numpy as np
import ml_dtypes

sys.path.insert(0, "/opt/trn_rl_repo")

import concourse.bass as bass
import concourse.mybir as mybir
from concourse.bass_utils import run_bass_kernel_spmd
from concourse.tile import TileContext

F32 = mybir.dt.float32
F16 = mybir.dt.float16
F8 = mybir.dt.float8e4
Alu = mybir.AluOpType
Act = mybir.ActivationFunctionType
E4M3 = ml_dtypes.float8_e4m3

# ---------------------------------------------------------------------------
# This container's walrus build only accepts a single sem-wait command per
# instruction; split multi-wait instructions into single-wait Drains.
# ---------------------------------------------------------------------------
_WAIT_LIMIT = 1


def _split_multi_waits(bir_bytes: bytes, limit: int = _WAIT_LIMIT) -> bytes:
    import orjson
    d = orjson.loads(bir_bytes)
    for fn in d.get("functions", []):
        for bb in fn.get("blocks", []):
            out = []
            for inst in bb.get("instructions", []):
                si = inst.get("sync_info") or {}
                waits = si.get("on_wait") or []
                if len(waits) > limit:
                    chunks = [waits[i:i + limit]
                              for i in range(0, len(waits), limit)]
                    for j, ch in enumerate(chunks[:-1]):
                        carrier = {
                            "engine": inst["engine"],
                            "ins": [],
                            "is_reset_sema": False,
                            "name": f"{inst['name']}__w{j}",
                            "opcode": "Drain",
                            "outs": [],
                            "sync_info": {"on_update": [], "on_wait": ch},
                        }
                        if "debug" in inst:
                            carrier["debug"] = inst["debug"]
                        out.append(carrier)
                    si["on_wait"] = chunks[-1]
                out.append(inst)
            bb["instructions"] = out
    return orjson.dumps(d)


def _install_wait_split_hook():
    from concourse import bass2jax
    orig = bass2jax.compile_bir_kernel
    if getattr(orig, "_wait_split_wrapped", False):
        return

    def wrapped(bir_bytes, *args, **kwargs):
        return orig(_split_multi_waits(bir_bytes), *args, **kwargs)

    wrapped._wait_split_wrapped = True
    bass2jax.compile_bir_kernel = wrapped


_install_wait_split_hook()

B, N, K = 128, 2048, 256
M = 8            # cores
BPC = B // M     # samples per core
KQ = K // 128    # contraction chunks
CAND = 32
HS = 32.0        # hi stream scale (keeps fp16 squares out of subnormals)
RS = 4096.0      # residual scale (pow2: RS*kv16 is exact in fp16)
GRP = 2          # samples per stream tile
SEG = N // 8     # 256: tail free size, 8 segments per sample


def build_nc():
    nc = bass.Bass()
    keysT_hi = nc.declare_dram_parameter("keysT_hi", [KQ, 128, BPC, N], F16, isOutput=False)
    keysT_res = nc.declare_dram_parameter("keysT_res", [KQ, 128, BPC, N], F8, isOutput=False)
    kvT17 = nc.declare_dram_parameter("kvT17", [KQ, 128, 32], F16, isOutput=False)
    kvT16r = nc.declare_dram_parameter("kvT16r", [KQ, 128, 32], F16, isOutput=False)
    lnscaleR = nc.declare_dram_parameter("lnscaleR", [128, 1], F32, isOutput=False)
    gammaR = nc.declare_dram_parameter("gammaR", [128, 1], F32, isOutput=False)
    rep16 = nc.declare_dram_parameter("rep16", [16, 128], F32, isOutput=False)
    rept = nc.declare_dram_parameter("rept", [128, 16], F32, isOutput=False)
    out = nc.declare_dram_parameter("out", [BPC, N], F32, isOutput=True)

    with TileContext(nc) as tc:
        with (
            tc.tile_pool(name="const", bufs=1) as cpool,
            tc.tile_pool(name="stream", bufs=3) as spool,
        ):
            kv, kvr = [], []
            for q in range(KQ):
                t = cpool.tile([128, 32], F16, name=f"kv{q}", tag=f"kv{q}")
                nc.sync.dma_start(out=t[:], in_=kvT17[q])
                kv.append(t)
                tr = cpool.tile([128, 32], F16, name=f"kvr{q}", tag=f"kvr{q}")
                nc.sync.dma_start(out=tr[:], in_=kvT16r[q])
                kvr.append(tr)
            lnsc_t = cpool.tile([128, 1], F32, tag="lnsc")
            nc.sync.dma_start(out=lnsc_t[:], in_=lnscaleR[:])
            gam_t = cpool.tile([128, 1], F32, tag="gam")
            nc.sync.dma_start(out=gam_t[:], in_=gammaR[:])
            rep16_t = cpool.tile([16, 128], F32, tag="rep16")
            nc.sync.dma_start(out=rep16_t[:], in_=rep16[:])
            rept_t = cpool.tile([128, 16], F32, tag="rept")
            nc.sync.dma_start(out=rept_t[:], in_=rept[:])

            # D/S in segment layout: sample b -> partitions 8b..8b+7
            Dseg = cpool.tile([128, SEG], F32, tag="Dseg")
            Sseg = cpool.tile([128, SEG], F32, tag="Sseg")

            # ---- stream: per group load hi+res, square hi, matmul jobs ----
            jobs = []
            for g in range(BPC // GRP):
                for j in range(GRP):
                    for kind in range(2):   # 0 = dots, 1 = sumsq
                        jobs.append((g, j, kind))

            with tc.tile_pool(name="psum", bufs=1, space="PSUM") as ppool:
                stream_tiles = {}
                cur = None
                cur_jobs = []
                slot = 3
                tile_i = 0

                def flush_tile():
                    # PSUM can't be DMA'd directly; stage via ACT/DVE copy
                    # (alternating to balance engine load), then row-DMA.
                    nrows = 32 * (len(cur_jobs) - 1) + 17
                    stag = spool.tile([96, N], F32, name=f"stag{tile_i}",
                                      tag="stag", bufs=2)
                    if tile_i % 2 == 0:
                        nc.vector.tensor_copy(stag[0:nrows, :],
                                              cur[0:nrows, :])
                    else:
                        nc.scalar.activation(stag[0:nrows, :],
                                             cur[0:nrows, :], Act.Copy)
                    for (s_, b_, kind_) in cur_jobs:
                        row = 32 * s_ + (b_ if kind_ == 0 else 16)
                        dst = Dseg if kind_ == 0 else Sseg
                        nc.sync.dma_start(out=dst[8 * b_:8 * b_ + 8, :],
                                          in_=stag[row:row + 1, :])

                for (g, j, kind) in jobs:
                    if g not in stream_tiles:
                        his, sqs, ress = [], [], []
                        for q in range(KQ):
                            hi = spool.tile([128, GRP * N], F16,
                                            name=f"hi{q}", tag=f"hi{q}", bufs=2)
                            nc.sync.dma_start(
                                out=hi[:],
                                in_=keysT_hi[q, :, GRP * g:GRP * (g + 1), :])
                            res = spool.tile([128, GRP * N], F8,
                                             name=f"res{q}", tag=f"res{q}",
                                             bufs=2)
                            nc.sync.dma_start(
                                out=res[:],
                                in_=keysT_res[q, :, GRP * g:GRP * (g + 1), :])
                            sq = spool.tile([128, GRP * N], F16,
                                            name=f"sq{q}", tag=f"sq{q}", bufs=2)
                            if (g + q) % 2 == 0:
                                nc.scalar.square(sq[:], hi[:])
                            else:
                                nc.vector.tensor_tensor(sq[:], hi[:], hi[:],
                                                        Alu.mult)
                            his.append(hi)
                            sqs.append(sq)
                            ress.append(res)
                        stream_tiles[g] = (his, sqs, ress)
                    his, sqs, ress = stream_tiles[g]
                    if slot == 3:
                        if cur is not None:
                            flush_tile()
                        tile_i += 1
                        cur = ppool.tile([96, N], F32, name=f"P{tile_i}",
                                         tag="P", bufs=2)
                        cur_jobs = []
                        slot = 0
                    for t in range(N // 512):
                        csl = slice(j * N + 512 * t, j * N + 512 * (t + 1))
                        po = cur[32 * slot:32 * slot + 32,
                                 512 * t:512 * (t + 1)]
                        if kind == 0:
                            for q in range(KQ):
                                nc.tensor.matmul(po, kv[q][:, 0:32],
                                                 his[q][:, csl],
                                                 start=(q == 0), stop=False)
                            for q in range(KQ):
                                nc.tensor.matmul(po, kvr[q][:, 0:32],
                                                 ress[q][:, csl],
                                                 start=False, stop=(q == KQ - 1))
                        else:
                            for q in range(KQ):
                                nc.tensor.matmul(po, kv[q][:, 0:32],
                                                 sqs[q][:, csl],
                                                 start=(q == 0),
                                                 stop=(q == KQ - 1))
                    cur_jobs.append((slot, GRP * g + j, kind))
                    slot += 1
                if cur is not None:
                    flush_tile()

            # ---- tail in segment layout [128, SEG] ----
            with tc.tile_pool(name="tpsum", bufs=1, space="PSUM") as tp:
                lnS = cpool.tile([128, SEG], F32, tag="t1", name="lnS")
                nc.scalar.activation(lnS[:], Sseg[:], Act.Ln)
                rfold = cpool.tile([128, SEG], F32, tag="t2", name="rfold")
                nc.scalar.activation(rfold[:], lnS[:], Act.Exp, scale=-0.5,
                                     bias=lnsc_t[:])
                logits = cpool.tile([128, SEG], F32, tag="t1", name="logits")
                nc.vector.tensor_tensor(logits[:], Dseg[:], rfold[:], Alu.mult)
                g1 = cpool.tile([128, SEG], F32, tag="t2", name="g1")
                nc.scalar.activation(g1[:], logits[:], Act.Exp, scale=gam_t[:])

                # per-segment top-32 (sorted desc) into cands[128, 32]
                e2 = cpool.tile([128, SEG], F32, tag="t3", name="e2")
                nc.vector.tensor_copy(e2[:], logits[:])
                cands = cpool.tile([128, 32], F32, tag="cands")
                for r in range(4):
                    nc.vector.max(cands[:, 8 * r:8 * r + 8], e2[:])
                    if r < 3:
                        nc.vector.match_replace(e2[:], cands[:, 8 * r:8 * r + 8],
                                                e2[:], -1e30)
                # gather to rows [16, 256] and take final top-32
                candsR = cpool.tile([16, 2 * 128], F32, tag="candsR")
                nc.sync.dma_start(out=candsR[:], in_=cands[:])
                m8f = cpool.tile([16, 8], F32, tag="m8f")
                for r in range(4):
                    nc.vector.max(m8f[:], candsR[:])
                    if r < 3:
                        nc.vector.match_replace(candsR[:], m8f[:],
                                                candsR[:], -1e30)
                # broadcast t32 [16,1] -> [128,1] via PE
                pt32 = tp.tile([128, 1], F32, tag="pt32")
                nc.tensor.matmul(pt32[:], rep16_t[:, 0:128], m8f[:, 7:8],
                                 start=True, stop=True)
                t32R = cpool.tile([128, 1], F32, tag="t32R")
                nc.vector.tensor_copy(t32R[:], pt32[:])

                # select + accumulate: etop = (logits >= t32) * g1
                etop = cpool.tile([128, SEG], F32, tag="t4", name="etop")
                zseg = cpool.tile([128, 1], F32, tag="zseg")
                nc.vector.scalar_tensor_tensor(
                    etop[:], logits[:], t32R[:], g1[:],
                    op0=Alu.is_ge, op1=Alu.mult, accum_out=zseg[:])
                # reduce seg sums to per-sample [16,1], recip, broadcast back
                pz = tp.tile([16, 1], F32, tag="pz")
                nc.tensor.matmul(pz[:], rept_t[:, 0:16], zseg[:],
                                 start=True, stop=True)
                zrow = cpool.tile([16, 1], F32, tag="zrow")
                nc.vector.tensor_copy(zrow[:], pz[:])
                zr = cpool.tile([16, 1], F32, tag="zr")
                nc.vector.reciprocal(zr[:], zrow[:])
                pzr = tp.tile([128, 1], F32, tag="pzr")
                nc.tensor.matmul(pzr[:], rep16_t[:, 0:128], zr[:],
                                 start=True, stop=True)
                zrR = cpool.tile([128, 1], F32, tag="zrR")
                nc.vector.tensor_copy(zrR[:], pzr[:])

                w = cpool.tile([128, SEG], F32, tag="t2", name="w")
                nc.vector.tensor_scalar(w[:], etop[:], zrR[:], None, Alu.mult)
                nc.sync.dma_start(out=out[:], in_=w[:])
    return nc


def shard_inputs(k, beta, gamma, keys):
    k = np.ascontiguousarray(k, dtype=np.float32)
    beta = np.ascontiguousarray(beta, dtype=np.float32)
    gamma = np.ascontiguousarray(gamma, dtype=np.float32)
    keys = np.ascontiguousarray(keys, dtype=np.float32)

    rep16 = np.zeros((16, 128), np.float32)
    rept = np.zeros((128, 16), np.float32)
    for s in range(16):
        rep16[s, 8 * s:8 * s + 8] = 1.0
        rept[8 * s:8 * s + 8, s] = 1.0

    in_maps = []
    for c in range(M):
        sl = slice(c * BPC, (c + 1) * BPC)
        kc = k[sl]                                   # [BPC, K] f32
        kvh16 = kc.astype(np.float16)
        kvh = kvh16.astype(np.float32)
        kvres = kc - kvh
        ratio = np.where(kvh != 0.0, kvres / np.where(kvh == 0.0, 1.0, kvh),
                         0.0).astype(np.float32)
        keysc = keys[sl]                             # [BPC, N, K] f32
        hi16 = (keysc * HS).astype(np.float16)
        hi = hi16.astype(np.float32)
        fold = (keysc * HS - hi) + hi * ratio[:, None, :]
        res8 = (fold * RS).astype(E4M3)

        keysT_hi = np.ascontiguousarray(
            hi16.transpose(2, 0, 1)).reshape(KQ, 128, BPC, N)
        keysT_res = np.ascontiguousarray(
            res8.transpose(2, 0, 1)).reshape(KQ, 128, BPC, N)

        kvT17 = np.zeros((KQ, 128, 32), np.float16)
        kvT17[:, :, 0:BPC] = (kvh16.astype(np.float32) * RS).astype(
            np.float16).T.reshape(KQ, 128, BPC)
        kvT17[:, :, 16] = 1.0
        kvT16r = np.zeros((KQ, 128, 32), np.float16)
        kvT16r[:, :, 0:BPC] = kvh16.T.reshape(KQ, 128, BPC)

        qn = np.maximum(np.linalg.norm(kc.astype(np.float64), axis=1), 1e-8)
        lnscale = (np.log(beta[sl].astype(np.float64)[:, 0]) - np.log(RS)
                   - np.log(qn)).astype(np.float32)
        lnscaleR = np.repeat(lnscale, 8)[:, None]            # [128, 1]
        gammaR = np.repeat(gamma[sl][:, 0], 8)[:, None].astype(np.float32)

        in_maps.append({
            "keysT_hi": keysT_hi,
            "keysT_res": keysT_res,
            "kvT17": kvT17,
            "kvT16r": kvT16r,
            "lnscaleR": np.ascontiguousarray(lnscaleR, dtype=np.float32),
            "gammaR": np.ascontiguousarray(gammaR, dtype=np.float32),
            "rep16": rep16,
            "rept": rept,
        })
    return in_maps


_NC_CACHE = None


def kernel(k=None, beta=None, gamma=None, keys=None, candidates=None, **_ignored):
    assert int(candidates) == CAND, f"kernel hardcoded for candidates=32, got {candidates}"
    assert keys.shape == (B, N, K), keys.shape
    global _NC_CACHE
    if _NC_CACHE is None:
        _NC_CACHE = build_nc()
    in_maps = shard_inputs(k, beta, gamma, keys)
    res = run_bass_kernel_spmd(_NC_CACHE, in_maps, list(range(M))).results
    return np.concatenate([res[c]["out"] for c in range(M)], axis=0)
